# revision 23
# baseline (speedup 1.0000x reference)
"""Trainium2 Bass kernel for nn_GCNLayer (3-layer GCN + max/mean pooling, T temporal slices).

Self-contained: hardcodes the problem shapes (N=50000, E=800000, B=250, T=8,
CIN=32, COUT=64) and distributes over 8 NeuronCores by graph/dst-node range.

v2 design (vs v1 baseline):
  - Layer-0 messages are pre-gathered on the HOST into edge-chunk order (xe)
    and streamed with plain sequential DMA - no gpsimd descriptor generation.
  - The one-hot scatter matrices are generated ON DEVICE per chunk via
    is_equal(dstl, iota) on the vector engine (0/1 entries), instead of
    loading ~34MB/layer of precomputed one-hots from HBM.
  - The symmetric norm dinv[src]*dinv[dst] is factored: rows stored in DRAM
    carry dinv[src] (host-baked for X, fused into the epilogue copy for H),
    and the aggregated block result is scaled by dinv[dst] (fused into the
    PSUM->SBUF copy).
  - Self-loops are not gathered: an identity matmul over the core's own
    contiguous rows adds them.
  - Per-(block,half) chunk counts (max over cores) instead of a uniform
    KL/KH, cutting padded descriptor count.
  - dma_gather calls round-robin over 4 SWDGE queues.
"""

import os
import numpy as np
import ml_dtypes

import concourse.bass as bass
import concourse.mybir as mybir
from concourse import bacc, tile
from concourse.bass_utils import run_bass_kernel_spmd

F32 = mybir.dt.float32
BF16 = mybir.dt.bfloat16
I16 = mybir.dt.int16
P = 128


class Cfg:
    def __init__(self, N=50000, E=800000, B=250, T=8, CIN=32, COUT=64,
                 NCORES=8, GRAPH=200):
        self.N, self.E, self.B, self.T = N, E, B, T
        self.CIN, self.COUT, self.NCORES, self.GRAPH = CIN, COUT, NCORES, GRAPH
        base, rem = divmod(B, NCORES)
        self.gpc = [base + (1 if c < rem else 0) for c in range(NCORES)]
        self.GPC = max(self.gpc)                      # uniform per-core graph slots
        self.NPC = self.GPC * GRAPH                   # padded nodes per core
        assert self.NPC % P == 0
        self.NBLK = self.NPC // P                     # dst blocks per core
        self.NPAD = self.NPC * NCORES                 # padded global node count
        self.HALF = self.NPAD // 2                    # gather index split point
        assert self.HALF <= 32767 + 1
        self.CH1 = CIN * T                            # layer-1 feature row
        self.CH = COUT * T                            # layer-2/3 feature row
        assert self.CH % P == 0
        self.NS = self.CH // P                        # psi partition tiles (t-pairs)
        self.GRP = 4 if self.NBLK >= 4 else self.NBLK
        self.SEGB = [0, 25, 40, 48, self.NBLK]    # segment block boundaries
        self.goff = np.concatenate([[0], np.cumsum(self.gpc)]).astype(np.int64)


def _wrap_idx_into(idx, ci, vals):
    """Write 128 int values for call ci chunk-position handled by caller.
    vals: flat positions i0..i0+127 handled by caller; here vals is (i0, arr)."""
    i0, arr = vals
    ii = i0 + np.arange(len(arr))
    idx[ii % 16, ci * 64 + ii // 16] = arr.astype(np.int16)


def preprocess(cfg, x, edge_index, batch, W1, b1, W2, b2, W3, b3):
    """Build all per-core device inputs. Returns (common, per_core_inputs, meta)."""
    N, E, T, CIN, COUT = cfg.N, cfg.E, cfg.T, cfg.CIN, cfg.COUT
    src = np.asarray(edge_index[0], np.int64)
    dst = np.asarray(edge_index[1], np.int64)

    # degrees incl self-loops, matching the reference
    deg = np.bincount(dst, minlength=N).astype(np.float32) + 1.0
    dinv = (1.0 / np.sqrt(deg)).astype(np.float32)

    batch = np.asarray(batch, np.int64)
    g2c = np.zeros(cfg.B, np.int64)
    for c in range(cfg.NCORES):
        g2c[cfg.goff[c]:cfg.goff[c + 1]] = c
    node_core = g2c[batch]
    first_node_of_core = np.array(
        [cfg.goff[c] * cfg.GRAPH for c in range(cfg.NCORES)], np.int64)
    local_n = np.arange(N) - first_node_of_core[node_core]
    CHK = cfg.NPC // 2

    # X' = dinv * X, row-major [N, T*CIN] (t-major rows)
    xm = np.moveaxis(np.asarray(x, np.float32), 2, 1).reshape(N, T * CIN)
    xs = xm * dinv[:, None]

    # 4 node segments by local block range; each gets its own rank-major
    # shared gather tensor and its own early AllGather.
    SEGB = np.asarray(cfg.SEGB, np.int64)
    NSEG = len(SEGB) - 1
    src_blk = local_n[src] // P
    src_seg = np.searchsorted(SEGB, src_blk, side="right") - 1
    seg_nb = np.diff(SEGB)
    srcp = (node_core[src] * (seg_nb[src_seg] * P)
            + (local_n[src] - SEGB[src_seg] * P))
    dstc = node_core[dst]
    dstl = local_n[dst]

    # ---- per-core edge bucketing (NO self loops)
    per_core = []
    cnt = np.zeros((cfg.NCORES, cfg.NBLK, NSEG), np.int64)
    for c in range(cfg.NCORES):
        m = dstc == c
        es, ed, so, sg = srcp[m], dstl[m], src[m], src_seg[m]
        blk = ed // P
        order = np.lexsort((es, sg, blk))
        es, ed, so, blk, sg = es[order], ed[order], so[order], blk[order], sg[order]
        per_core.append((es, ed, so, blk, sg))
        for b in range(cfg.NBLK):
            mb = blk == b
            for s in range(NSEG):
                cnt[c, b, s] = int(((sg == s) & mb).sum())

    # per-(block,seg) chunk count = max over cores
    K = -(-cnt // P)            # ceil
    Kbh = K.max(axis=0)         # [NBLK, NSEG]

    groups = []
    b0 = 0
    while b0 < cfg.NBLK:
        groups.append(list(range(b0, min(b0 + cfg.GRP, cfg.NBLK))))
        b0 += cfg.GRP

    # call table: (half, group, pos, k) with chunk owner lists
    calls = []
    chunk_map = {}              # (h, gi, chunk_pos_in_group) -> (ci, j)
    owners = {}                 # (h, gi) -> list of (bi, k_index) per chunk pos
    for h in range(NSEG):
        for gi, blks in enumerate(groups):
            own = []
            for bi, b in enumerate(blks):
                for k in range(int(Kbh[b, h])):
                    own.append((bi, k))
            owners[(h, gi)] = own
            pos = 0
            while pos < len(own):
                k = min(8, len(own) - pos)
                ci = len(calls)
                calls.append((h, gi, pos, k))
                for j in range(k):
                    chunk_map[(h, gi, pos + j)] = (ci, j)
                pos += k
            if not own:
                pass
    NCALLS = len(calls)

    # ---- per-core tensors
    per_core_inputs = []
    for c in range(cfg.NCORES):
        es, ed, so, blk, sg = per_core[c]
        gidx = np.zeros((16, NCALLS * 64), np.int16)
        dstl_t = np.full((P, NCALLS * 8), 200.0, np.float32)
        xe = np.zeros((NCALLS, P, 8 * cfg.CH1), ml_dtypes.bfloat16)
        # slice edges per (b, h)
        starts = {}
        i0 = 0
        for b in range(cfg.NBLK):
            for h in range(NSEG):
                n_e = int(cnt[c, b, h])
                starts[(b, h)] = i0
                i0 += n_e
        for h in range(NSEG):
            for gi, blks in enumerate(groups):
                own = owners[(h, gi)]
                for cpos, (bi, k) in enumerate(own):
                    b = blks[bi]
                    ci, j = chunk_map[(h, gi, cpos)]
                    e0 = starts[(b, h)] + k * P
                    e1 = min(starts[(b, h)] + int(cnt[c, b, h]), e0 + P)
                    ne = max(0, e1 - e0)
                    vals = np.zeros(P, np.int64)
                    if ne:
                        vals[:ne] = es[e0:e1]
                        dstl_t[:ne, ci * 8 + j] = (ed[e0:e1] - b * P).astype(np.float32)
                        xe[ci, :ne, j * cfg.CH1:(j + 1) * cfg.CH1] = \
                            xs[so[e0:e1]].astype(ml_dtypes.bfloat16)
                    ii = j * P + np.arange(P)
                    gidx[ii % 16, ci * 64 + ii // 16] = vals.astype(np.int16)
        gidx = np.tile(gidx, (8, 1))

        # core's own X' rows in local order (self-loop rhs), pad rows zero
        nreal = cfg.gpc[c] * cfg.GRAPH
        xself = np.zeros((cfg.NPC, cfg.CH1), np.float32)
        xself[:nreal] = xs[first_node_of_core[c]:first_node_of_core[c] + nreal]

        # dinv per local node as [128, NBLK]
        dv = np.zeros(cfg.NPC, np.float32)
        dv[:nreal] = dinv[first_node_of_core[c]:first_node_of_core[c] + nreal]
        dinvcol = dv.reshape(cfg.NBLK, P).T.copy()

        per_core_inputs.append({
            "gidx": gidx,
            "dstl": dstl_t.astype(ml_dtypes.bfloat16),
            "xe": xe,
            "xself": xself.astype(ml_dtypes.bfloat16),
            "dinvcol": dinvcol,
        })

    # weights: zero-padded [128,128] lhsT variants
    wz = np.zeros((12, P, P), np.float32)
    for li, W in enumerate((W1, W2, W3)):
        W = np.asarray(W, np.float32)
        kdim = W.shape[0]
        nq = P // kdim
        for q in range(nq):
            hh = q % 2
            wz[li * 4 + q, q * kdim:(q + 1) * kdim, hh * COUT:(hh + 1) * COUT] = W

    bias_col = np.zeros((P, 3), np.float32)
    for i, b in enumerate((b1, b2, b3)):
        bias_col[:, i] = np.tile(np.asarray(b, np.float32), P // COUT)

    ident = np.eye(P, dtype=np.float32)
    iota = np.tile(np.arange(P, dtype=np.float32)[None, :], (P, 8))

    # pooling piece table per group
    pool_pieces = []
    seen = set()
    for gi, blks in enumerate(groups):
        n0g, n1g = blks[0] * P, (blks[-1] + 1) * P
        pieces = []
        n = n0g
        while n < n1g:
            gl = n // cfg.GRAPH
            nend = min((gl + 1) * cfg.GRAPH, n1g)
            ft = gl not in seen
            seen.add(gl)
            pieces.append((n - n0g, nend - n0g, gl, ft))
            n = nend
        pool_pieces.append(pieces)

    common = {
        "wz": wz.astype(ml_dtypes.bfloat16),
        "biascol": bias_col,
        "id_f32": ident,
        "id_bf": ident.astype(ml_dtypes.bfloat16),
        "iota_bf": iota.astype(ml_dtypes.bfloat16),
    }
    meta = dict(Kbh=Kbh, calls=calls, chunk_map=chunk_map, owners=owners,
                groups=groups, pool_pieces=pool_pieces, NCALLS=NCALLS)
    return common, per_core_inputs, meta


def build(cfg, meta):
    calls, chunk_map, owners = meta["calls"], meta["chunk_map"], meta["owners"]
    groups, pool_pieces, NCALLS = meta["groups"], meta["pool_pieces"], meta["NCALLS"]
    NS, CH, CH1, T, COUT, CIN = cfg.NS, cfg.CH, cfg.CH1, cfg.T, cfg.COUT, cfg.CIN
    NS1 = max(CH1 // P, 1)

    nc = bacc.Bacc("TRN2", target_bir_lowering=False, debug=False,
                   num_devices=cfg.NCORES, num_swdge_queues=4)

    gidx_d = nc.dram_tensor("gidx", [P, NCALLS * 64], I16, kind="ExternalInput")
    dstl_d = nc.dram_tensor("dstl", [P, NCALLS * 8], BF16, kind="ExternalInput")
    xe_d = nc.dram_tensor("xe", [NCALLS, P, 8 * CH1], BF16, kind="ExternalInput")
    xself_d = nc.dram_tensor("xself", [cfg.NPC, CH1], BF16, kind="ExternalInput")
    dinv_d = nc.dram_tensor("dinvcol", [P, cfg.NBLK], F32, kind="ExternalInput")
    wz_d = nc.dram_tensor("wz", [12, P, P], BF16, kind="ExternalInput")
    biascol = nc.dram_tensor("biascol", [P, 3], F32, kind="ExternalInput")
    id_f32 = nc.dram_tensor("id_f32", [P, P], F32, kind="ExternalInput")
    id_bf = nc.dram_tensor("id_bf", [P, P], BF16, kind="ExternalInput")
    iota_d = nc.dram_tensor("iota_bf", [P, 8 * P], BF16, kind="ExternalInput")
    out = nc.dram_tensor("out", [P, 2 * NS * cfg.GPC], F32, kind="ExternalOutput")

    rg = [list(range(cfg.NCORES))]

    with tile.TileContext(nc) as tc:
        with (
            tc.tile_pool(name="const", bufs=1) as constp,
            tc.tile_pool(name="msg", bufs=3) as msgp,
            tc.tile_pool(name="msgh", bufs=3) as msghp,
            tc.tile_pool(name="otile", bufs=3) as otp,
            tc.tile_pool(name="oth", bufs=3) as othp,
            tc.tile_pool(name="selfr", bufs=3) as selfp,
            tc.tile_pool(name="work", bufs=2) as workp,
            tc.tile_pool(name="psig", bufs=2) as psigp,
            tc.tile_pool(name="pool", bufs=1) as poolp,
            tc.tile_pool(name="gps", bufs=4, space="PSUM") as gpsp,
            tc.tile_pool(name="t1ps", bufs=1, space="PSUM") as t1psp,
            tc.tile_pool(name="psips", bufs=2, space="PSUM") as psipsp,
            tc.tile_pool(name="t2ps", bufs=1, space="PSUM") as t2psp,
            tc.tile_pool(name="dram", bufs=1, space="DRAM") as dramp,
        ):
            # ---- constants into SBUF
            idx_sb = constp.tile([P, NCALLS * 64], I16)
            nc.sync.dma_start(out=idx_sb[:], in_=gidx_d[:])
            dstl_sb = constp.tile([P, NCALLS * 8], BF16)
            nc.sync.dma_start(out=dstl_sb[:], in_=dstl_d[:])
            wzt = constp.tile([P, 12 * P], BF16, tag="wzt")
            nc.sync.dma_start(
                out=wzt[:].rearrange("p (i m) -> p i m", i=12),
                in_=wz_d.ap().rearrange("i p m -> p i m"))
            bct = constp.tile([P, 3], F32)
            nc.sync.dma_start(out=bct[:], in_=biascol[:])
            idf = constp.tile([P, P], F32)
            nc.sync.dma_start(out=idf[:], in_=id_f32[:])
            idb = constp.tile([P, P], BF16)
            nc.sync.dma_start(out=idb[:], in_=id_bf[:])
            iota_sb = constp.tile([P, 8 * P], BF16)
            nc.sync.dma_start(out=iota_sb[:], in_=iota_d[:])
            dinv_sb = constp.tile([P, cfg.NBLK], F32)
            nc.sync.dma_start(out=dinv_sb[:], in_=dinv_d[:])

            # ---- pool accumulators
            lmax = poolp.tile([P, NS * cfg.GPC], F32, tag="lmax")
            lsum = poolp.tile([P, NS * cfg.GPC], F32, tag="lsum")
            fmax = poolp.tile([P, NS * cfg.GPC], F32, tag="fmax")
            fsum = poolp.tile([P, NS * cfg.GPC], F32, tag="fsum")
            for _t in (lmax, lsum, fmax, fsum):
                nc.vector.memset(_t[:], 0.0)

            # ---- DRAM intermediates
            SEGB = cfg.SEGB
            NSEG = len(SEGB) - 1
            h_mine, h_full = [], []
            for i in range(2):
                hm = dramp.tile([cfg.NPC, CH], BF16, tag=f"hm{i}")
                h_mine.append(hm)
                segs = []
                for s in range(NSEG):
                    nb = SEGB[s + 1] - SEGB[s]
                    hfs = dramp.tile([cfg.NCORES * nb * P, CH], BF16,
                                     tag=f"hf{i}s{s}", addr_space="Shared")
                    segs.append(hfs)
                h_full.append(segs)

            def layer(li):
                ch_in = CH1 if li == 0 else CH
                ns_in = NS1 if li == 0 else NS
                if li > 0:
                    src_seg = [h[:, :] for h in h_full[li - 1]]

                calls_of_group = {}
                for ci, (h, gi, pos, k) in enumerate(calls):
                    calls_of_group.setdefault(gi, []).append((ci, h, pos, k))

                for gi, blks in enumerate(groups):
                    gtiles = {}
                    for ci, h, pos, k in calls_of_group.get(gi, []):
                        ni = k * P
                        g = (msgp if h < 2 else msghp).tile(
                            [P, 8 * ch_in], BF16, tag=f"m{h}")
                        if li == 0:
                            nc.sync.dma_start(out=g[:, :k * ch_in],
                                              in_=xe_d[ci, :, :k * ch_in])
                        else:
                            nc.gpsimd.dma_gather(
                                out_ap=g[:, :k * ch_in].rearrange(
                                    "p (c e) -> p c e", e=ch_in),
                                in_ap=src_seg[h],
                                idxs_ap=idx_sb[:, ci * 64: ci * 64 + max(ni // 16, 1)],
                                num_idxs=ni,
                                num_idxs_reg=ni,
                                elem_size=ch_in,
                                queue_num=ci % 4,
                            )
                        ot = (otp if h < 2 else othp).tile([P, 8 * P], BF16,
                                                           tag=f"oo{h}")
                        nc.vector.tensor_tensor(
                            out=ot[:, :k * P].rearrange("p (c d) -> p c d", d=P),
                            in0=dstl_sb[:, ci * 8: ci * 8 + k
                                        ].to_broadcast([P, k, P]),
                            in1=iota_sb[:, :k * P].rearrange(
                                "p (c d) -> p c d", d=P),
                            op=mybir.AluOpType.is_equal)
                        gtiles[ci] = (g, ot)

                    psi_grp = psigp.tile([P, NS * len(blks) * P], F32, tag="psig")
                    for bi, b in enumerate(blks):
                        # self-loop rhs: own rows (X' for L0, H' for L1/2)
                        selfr = selfp.tile([P, ch_in], BF16, tag="selfr")
                        if li == 0:
                            nc.sync.dma_start(out=selfr[:],
                                              in_=xself_d[b * P:(b + 1) * P, :])
                        else:
                            nc.sync.dma_start(
                                out=selfr[:],
                                in_=h_mine[li - 1][b * P:(b + 1) * P, :])
                        # chunks of this block
                        mychunks = []
                        for h in range(NSEG):
                            own = owners[(h, gi)]
                            for cpos, (obi, k) in enumerate(own):
                                if obi == bi:
                                    mychunks.append(chunk_map[(h, gi, cpos)])
                        gps = gpsp.tile([P, ch_in], F32, tag="gps")
                        nc.tensor.matmul(gps[:], lhsT=idb[:], rhs=selfr[:],
                                         start=True, stop=(len(mychunks) == 0))
                        for mi, (ci, j) in enumerate(mychunks):
                            g, ot = gtiles[ci]
                            nc.tensor.matmul(
                                gps[:],
                                lhsT=ot[:, j * P:(j + 1) * P],
                                rhs=g[:, j * ch_in:(j + 1) * ch_in],
                                start=False, stop=(mi == len(mychunks) - 1),
                            )
                        # ---- epilogue for block b
                        gbf = workp.tile([P, ch_in], F32, tag="gbf")
                        nc.scalar.activation(gbf[:], gps[:],
                                             mybir.ActivationFunctionType.Copy,
                                             scale=dinv_sb[:, b:b + 1])
                        t1 = t1psp.tile([P, ns_in * P], F32, tag="t1")
                        for s in range(ns_in):
                            nc.tensor.transpose(
                                t1[:, s * P:(s + 1) * P],
                                gbf[:, s * P:(s + 1) * P], idf[:])
                        gt = workp.tile([P, ns_in * P], BF16, tag="gt")
                        nc.scalar.activation(gt[:], t1[:],
                                             mybir.ActivationFunctionType.Copy)
                        psi_ps = psipsp.tile([P, NS * P], F32, tag="psip")
                        kdim = CIN if li == 0 else COUT
                        nq = P // kdim
                        for t_ in range(T):
                            s_out = t_ // 2
                            q = t_ % nq
                            s_in = t_ // nq
                            nc.tensor.matmul(
                                psi_ps[:, s_out * P:(s_out + 1) * P],
                                lhsT=wzt[:, (li * 4 + q) * P:(li * 4 + q + 1) * P],
                                rhs=gt[:, s_in * P:(s_in + 1) * P],
                                start=(t_ % 2 == 0), stop=(t_ % 2 == 1),
                            )
                        gwk = len(blks) * P
                        dst_view = psi_grp[:].rearrange(
                            "p (s n) -> p s n", n=gwk)[:, :, bi * P:(bi + 1) * P]
                        nc.scalar.activation(
                            dst_view,
                            psi_ps[:].rearrange("p (s n) -> p s n", s=NS),
                            mybir.ActivationFunctionType.Relu,
                            bias=bct[:, li:li + 1],
                        )
                        if li < 2:
                            t2 = t2psp.tile([P, NS * P], F32, tag="t2")
                            for s in range(NS):
                                nc.tensor.transpose(
                                    t2[:, s * P:(s + 1) * P],
                                    psi_grp[:, s * len(blks) * P + bi * P:
                                            s * len(blks) * P + (bi + 1) * P],
                                    idf[:])
                            hbf = workp.tile([P, CH], BF16, tag="hbf")
                            nc.vector.tensor_scalar_mul(hbf[:], t2[:],
                                                        dinv_sb[:, b:b + 1])
                            nc.sync.dma_start(
                                out=h_mine[li][b * P:(b + 1) * P, :], in_=hbf[:])

                    # ---- pooling for this group
                    gw = len(blks) * P
                    for s in range(NS):
                        base = s * gw
                        for (n0, n1, gl, ft) in pool_pieces[gi]:
                            seg = psi_grp[:, base + n0: base + n1]
                            if ft:
                                nc.vector.reduce_max(
                                    out=lmax[:, s * cfg.GPC + gl: s * cfg.GPC + gl + 1],
                                    in_=seg, axis=mybir.AxisListType.X)
                                nc.vector.reduce_sum(
                                    out=lsum[:, s * cfg.GPC + gl: s * cfg.GPC + gl + 1],
                                    in_=seg, axis=mybir.AxisListType.X)
                            else:
                                tm = workp.tile([P, 2], F32, tag="ptmp")
                                nc.vector.reduce_max(out=tm[:, 0:1], in_=seg,
                                                     axis=mybir.AxisListType.X)
                                nc.vector.reduce_sum(out=tm[:, 1:2], in_=seg,
                                                     axis=mybir.AxisListType.X)
                                nc.vector.tensor_tensor(
                                    out=lmax[:, s * cfg.GPC + gl: s * cfg.GPC + gl + 1],
                                    in0=lmax[:, s * cfg.GPC + gl: s * cfg.GPC + gl + 1],
                                    in1=tm[:, 0:1], op=mybir.AluOpType.max)
                                nc.vector.tensor_add(
                                    out=lsum[:, s * cfg.GPC + gl: s * cfg.GPC + gl + 1],
                                    in0=lsum[:, s * cfg.GPC + gl: s * cfg.GPC + gl + 1],
                                    in1=tm[:, 1:2])

                    # progressive allgathers: each segment fires as soon as
                    # its blocks are done; the last segment is tiny, so the
                    # layer-boundary stall is small.
                    if li < 2 and int(os.environ.get("GCN_LAYERS", "3")) > li + 1:
                        for s in range(NSEG - 1):
                            if gi == (SEGB[s + 1] - 1) // cfg.GRP:
                                nc.gpsimd.collective_compute(
                                    "AllGather", mybir.AluOpType.bypass,
                                    replica_groups=rg,
                                    ins=[h_mine[li][SEGB[s] * P:
                                                    SEGB[s + 1] * P, :]],
                                    outs=[h_full[li][s][:, :]],
                                )

                # ---- layer end: accumulate pools
                if li == 0:
                    nc.vector.tensor_copy(out=fmax[:], in_=lmax[:])
                    nc.vector.tensor_copy(out=fsum[:], in_=lsum[:])
                else:
                    nc.vector.tensor_add(out=fmax[:], in0=fmax[:], in1=lmax[:])
                    nc.vector.tensor_add(out=fsum[:], in0=fsum[:], in1=lsum[:])

                if li < 2 and int(os.environ.get("GCN_LAYERS", "3")) > li + 1:
                    s = NSEG - 1
                    nc.gpsimd.collective_compute(
                        "AllGather", mybir.AluOpType.bypass,
                        replica_groups=rg,
                        ins=[h_mine[li][SEGB[s] * P:SEGB[s + 1] * P, :]],
                        outs=[h_full[li][s][:, :]],
                    )

            for _li in range(int(os.environ.get("GCN_LAYERS", "3"))):
                layer(_li)

            nc.vector.tensor_scalar_mul(fsum[:], fsum[:],
                                        float(np.float32(1.0 / cfg.GRAPH)))
            osb = workp.tile([P, 2 * NS * cfg.GPC], F32, tag="osb")
            nc.vector.tensor_copy(out=osb[:, :NS * cfg.GPC], in_=fmax[:])
            nc.vector.tensor_copy(out=osb[:, NS * cfg.GPC:], in_=fsum[:])
            nc.sync.dma_start(out=out[:], in_=osb[:])

    nc.compile()
    return nc


def unshard(cfg, results):
    """[NCORES][128, 2*NS*GPC] -> [B, 2*COUT, T] float32."""
    B, T, COUT, NS, GPC = cfg.B, cfg.T, cfg.COUT, cfg.NS, cfg.GPC
    out = np.zeros((B, 2 * COUT, T), np.float32)
    for c in range(cfg.NCORES):
        V = results[c]["out"]
        for gl in range(cfg.gpc[c]):
            g = cfg.goff[c] + gl
            for s in range(NS):
                for half in range(2):
                    t_ = 2 * s + half
                    co = np.arange(COUT)
                    pp = half * COUT + co
                    out[g, co, t_] = V[pp, s * GPC + gl]
                    out[g, COUT + co, t_] = V[pp, NS * GPC + s * GPC + gl]
    return out


_CACHE = {}


def kernel(**inputs):
    cfg = Cfg()
    common, per_core, meta = preprocess(
        cfg, inputs["x"], inputs["edge_index"], inputs["batch"],
        inputs["W1"], inputs["b1"], inputs["W2"], inputs["b2"],
        inputs["W3"], inputs["b3"])
    key = (tuple(meta["Kbh"].flatten().tolist()), meta["NCALLS"])
    if key not in _CACHE:
        _CACHE[key] = build(cfg, meta)
    nc = _CACHE[key]
    in_maps = []
    for c in range(cfg.NCORES):
        m = dict(common)
        m.update(per_core[c])
        in_maps.append(m)
    res = run_bass_kernel_spmd(nc, in_maps, list(range(cfg.NCORES)))
    return unshard(cfg, res.results)


# revision 24
# speedup vs baseline: 1.1178x; 1.1178x over previous
"""Trainium2 Bass kernel for nn_GCNLayer (3-layer GCN + max/mean pooling, T temporal slices).

Self-contained: hardcodes the problem shapes (N=50000, E=800000, B=250, T=8,
CIN=32, COUT=64) and distributes over 8 NeuronCores by graph/dst-node range.

v2 design (vs v1 baseline):
  - Layer-0 messages are pre-gathered on the HOST into edge-chunk order (xe)
    and streamed with plain sequential DMA - no gpsimd descriptor generation.
  - The one-hot scatter matrices are generated ON DEVICE per chunk via
    is_equal(dstl, iota) on the vector engine (0/1 entries), instead of
    loading ~34MB/layer of precomputed one-hots from HBM.
  - The symmetric norm dinv[src]*dinv[dst] is factored: rows stored in DRAM
    carry dinv[src] (host-baked for X, fused into the epilogue copy for H),
    and the aggregated block result is scaled by dinv[dst] (fused into the
    PSUM->SBUF copy).
  - Self-loops are not gathered: an identity matmul over the core's own
    contiguous rows adds them.
  - Per-(block,half) chunk counts (max over cores) instead of a uniform
    KL/KH, cutting padded descriptor count.
  - dma_gather calls round-robin over 4 SWDGE queues.
"""

import os
import numpy as np
import ml_dtypes

import concourse.bass as bass
import concourse.mybir as mybir
from concourse import bacc, tile
from concourse.bass_utils import run_bass_kernel_spmd

F32 = mybir.dt.float32
BF16 = mybir.dt.bfloat16
I16 = mybir.dt.int16
P = 128


class Cfg:
    def __init__(self, N=50000, E=800000, B=250, T=8, CIN=32, COUT=64,
                 NCORES=8, GRAPH=200):
        self.N, self.E, self.B, self.T = N, E, B, T
        self.CIN, self.COUT, self.NCORES, self.GRAPH = CIN, COUT, NCORES, GRAPH
        base, rem = divmod(B, NCORES)
        self.gpc = [base + (1 if c < rem else 0) for c in range(NCORES)]
        self.GPC = max(self.gpc)                      # uniform per-core graph slots
        self.NPC = self.GPC * GRAPH                   # padded nodes per core
        assert self.NPC % P == 0
        self.NBLK = self.NPC // P                     # dst blocks per core
        self.NPAD = self.NPC * NCORES                 # padded global node count
        self.HALF = self.NPAD // 2                    # gather index split point
        assert self.HALF <= 32767 + 1
        self.CH1 = CIN * T                            # layer-1 feature row
        self.CH = COUT * T                            # layer-2/3 feature row
        assert self.CH % P == 0
        self.NS = self.CH // P                        # psi partition tiles (t-pairs)
        self.GRP = 4 if self.NBLK >= 4 else self.NBLK
        self.SEGB = [0, 25, self.NBLK]    # segment block boundaries
        self.goff = np.concatenate([[0], np.cumsum(self.gpc)]).astype(np.int64)


def _wrap_idx_into(idx, ci, vals):
    """Write 128 int values for call ci chunk-position handled by caller.
    vals: flat positions i0..i0+127 handled by caller; here vals is (i0, arr)."""
    i0, arr = vals
    ii = i0 + np.arange(len(arr))
    idx[ii % 16, ci * 64 + ii // 16] = arr.astype(np.int16)


def preprocess(cfg, x, edge_index, batch, W1, b1, W2, b2, W3, b3):
    """Build all per-core device inputs. Returns (common, per_core_inputs, meta)."""
    N, E, T, CIN, COUT = cfg.N, cfg.E, cfg.T, cfg.CIN, cfg.COUT
    src = np.asarray(edge_index[0], np.int64)
    dst = np.asarray(edge_index[1], np.int64)

    # degrees incl self-loops, matching the reference
    deg = np.bincount(dst, minlength=N).astype(np.float32) + 1.0
    dinv = (1.0 / np.sqrt(deg)).astype(np.float32)

    batch = np.asarray(batch, np.int64)
    g2c = np.zeros(cfg.B, np.int64)
    for c in range(cfg.NCORES):
        g2c[cfg.goff[c]:cfg.goff[c + 1]] = c
    node_core = g2c[batch]
    first_node_of_core = np.array(
        [cfg.goff[c] * cfg.GRAPH for c in range(cfg.NCORES)], np.int64)
    local_n = np.arange(N) - first_node_of_core[node_core]
    CHK = cfg.NPC // 2

    # X' = dinv * X, row-major [N, T*CIN] (t-major rows)
    xm = np.moveaxis(np.asarray(x, np.float32), 2, 1).reshape(N, T * CIN)
    xs = xm * dinv[:, None]

    # 4 node segments by local block range; each gets its own rank-major
    # shared gather tensor and its own early AllGather.
    SEGB = np.asarray(cfg.SEGB, np.int64)
    NSEG = len(SEGB) - 1
    src_blk = local_n[src] // P
    src_seg = np.searchsorted(SEGB, src_blk, side="right") - 1
    seg_nb = np.diff(SEGB)
    srcp = (node_core[src] * (seg_nb[src_seg] * P)
            + (local_n[src] - SEGB[src_seg] * P))
    dstc = node_core[dst]
    dstl = local_n[dst]

    # ---- per-core edge bucketing (NO self loops)
    per_core = []
    cnt = np.zeros((cfg.NCORES, cfg.NBLK, NSEG), np.int64)
    for c in range(cfg.NCORES):
        m = dstc == c
        es, ed, so, sg = srcp[m], dstl[m], src[m], src_seg[m]
        blk = ed // P
        order = np.lexsort((es, sg, blk))
        es, ed, so, blk, sg = es[order], ed[order], so[order], blk[order], sg[order]
        per_core.append((es, ed, so, blk, sg))
        for b in range(cfg.NBLK):
            mb = blk == b
            for s in range(NSEG):
                cnt[c, b, s] = int(((sg == s) & mb).sum())

    # per-(block,seg) chunk count = max over cores
    K = -(-cnt // P)            # ceil
    Kbh = K.max(axis=0)         # [NBLK, NSEG]

    groups = []
    b0 = 0
    while b0 < cfg.NBLK:
        groups.append(list(range(b0, min(b0 + cfg.GRP, cfg.NBLK))))
        b0 += cfg.GRP

    # call table: (half, group, pos, k) with chunk owner lists
    calls = []
    chunk_map = {}              # (h, gi, chunk_pos_in_group) -> (ci, j)
    owners = {}                 # (h, gi) -> list of (bi, k_index) per chunk pos
    for h in range(NSEG):
        for gi, blks in enumerate(groups):
            own = []
            for bi, b in enumerate(blks):
                for k in range(int(Kbh[b, h])):
                    own.append((bi, k))
            owners[(h, gi)] = own
            pos = 0
            while pos < len(own):
                k = min(8, len(own) - pos)
                ci = len(calls)
                calls.append((h, gi, pos, k))
                for j in range(k):
                    chunk_map[(h, gi, pos + j)] = (ci, j)
                pos += k
            if not own:
                pass
    NCALLS = len(calls)

    # ---- per-core tensors
    per_core_inputs = []
    for c in range(cfg.NCORES):
        es, ed, so, blk, sg = per_core[c]
        gidx = np.zeros((16, NCALLS * 64), np.int16)
        dstl_t = np.full((P, NCALLS * 8), 200.0, np.float32)
        xe = np.zeros((NCALLS, P, 8 * cfg.CH1), ml_dtypes.bfloat16)
        # slice edges per (b, h)
        starts = {}
        i0 = 0
        for b in range(cfg.NBLK):
            for h in range(NSEG):
                n_e = int(cnt[c, b, h])
                starts[(b, h)] = i0
                i0 += n_e
        for h in range(NSEG):
            for gi, blks in enumerate(groups):
                own = owners[(h, gi)]
                for cpos, (bi, k) in enumerate(own):
                    b = blks[bi]
                    ci, j = chunk_map[(h, gi, cpos)]
                    e0 = starts[(b, h)] + k * P
                    e1 = min(starts[(b, h)] + int(cnt[c, b, h]), e0 + P)
                    ne = max(0, e1 - e0)
                    vals = np.zeros(P, np.int64)
                    if ne:
                        vals[:ne] = es[e0:e1]
                        dstl_t[:ne, ci * 8 + j] = (ed[e0:e1] - b * P).astype(np.float32)
                        xe[ci, :ne, j * cfg.CH1:(j + 1) * cfg.CH1] = \
                            xs[so[e0:e1]].astype(ml_dtypes.bfloat16)
                    ii = j * P + np.arange(P)
                    gidx[ii % 16, ci * 64 + ii // 16] = vals.astype(np.int16)
        gidx = np.tile(gidx, (8, 1))

        # core's own X' rows in local order (self-loop rhs), pad rows zero
        nreal = cfg.gpc[c] * cfg.GRAPH
        xself = np.zeros((cfg.NPC, cfg.CH1), np.float32)
        xself[:nreal] = xs[first_node_of_core[c]:first_node_of_core[c] + nreal]

        # dinv per local node as [128, NBLK]
        dv = np.zeros(cfg.NPC, np.float32)
        dv[:nreal] = dinv[first_node_of_core[c]:first_node_of_core[c] + nreal]
        dinvcol = dv.reshape(cfg.NBLK, P).T.copy()

        per_core_inputs.append({
            "gidx": gidx,
            "dstl": dstl_t.astype(ml_dtypes.bfloat16),
            "xe": xe,
            "xself": xself.astype(ml_dtypes.bfloat16),
            "dinvcol": dinvcol,
        })

    # weights: zero-padded [128,128] lhsT variants
    wz = np.zeros((12, P, P), np.float32)
    for li, W in enumerate((W1, W2, W3)):
        W = np.asarray(W, np.float32)
        kdim = W.shape[0]
        nq = P // kdim
        for q in range(nq):
            hh = q % 2
            wz[li * 4 + q, q * kdim:(q + 1) * kdim, hh * COUT:(hh + 1) * COUT] = W

    bias_col = np.zeros((P, 3), np.float32)
    for i, b in enumerate((b1, b2, b3)):
        bias_col[:, i] = np.tile(np.asarray(b, np.float32), P // COUT)

    ident = np.eye(P, dtype=np.float32)
    iota = np.tile(np.arange(P, dtype=np.float32)[None, :], (P, 8))

    # pooling piece table per group
    pool_pieces = []
    seen = set()
    for gi, blks in enumerate(groups):
        n0g, n1g = blks[0] * P, (blks[-1] + 1) * P
        pieces = []
        n = n0g
        while n < n1g:
            gl = n // cfg.GRAPH
            nend = min((gl + 1) * cfg.GRAPH, n1g)
            ft = gl not in seen
            seen.add(gl)
            pieces.append((n - n0g, nend - n0g, gl, ft))
            n = nend
        pool_pieces.append(pieces)

    common = {
        "wz": wz.astype(ml_dtypes.bfloat16),
        "biascol": bias_col,
        "id_f32": ident,
        "id_bf": ident.astype(ml_dtypes.bfloat16),
        "iota_bf": iota.astype(ml_dtypes.bfloat16),
    }
    meta = dict(Kbh=Kbh, calls=calls, chunk_map=chunk_map, owners=owners,
                groups=groups, pool_pieces=pool_pieces, NCALLS=NCALLS)
    return common, per_core_inputs, meta


def build(cfg, meta):
    calls, chunk_map, owners = meta["calls"], meta["chunk_map"], meta["owners"]
    groups, pool_pieces, NCALLS = meta["groups"], meta["pool_pieces"], meta["NCALLS"]
    NS, CH, CH1, T, COUT, CIN = cfg.NS, cfg.CH, cfg.CH1, cfg.T, cfg.COUT, cfg.CIN
    NS1 = max(CH1 // P, 1)

    nc = bacc.Bacc("TRN2", target_bir_lowering=False, debug=False,
                   num_devices=cfg.NCORES, num_swdge_queues=4)

    gidx_d = nc.dram_tensor("gidx", [P, NCALLS * 64], I16, kind="ExternalInput")
    dstl_d = nc.dram_tensor("dstl", [P, NCALLS * 8], BF16, kind="ExternalInput")
    xe_d = nc.dram_tensor("xe", [NCALLS, P, 8 * CH1], BF16, kind="ExternalInput")
    xself_d = nc.dram_tensor("xself", [cfg.NPC, CH1], BF16, kind="ExternalInput")
    dinv_d = nc.dram_tensor("dinvcol", [P, cfg.NBLK], F32, kind="ExternalInput")
    wz_d = nc.dram_tensor("wz", [12, P, P], BF16, kind="ExternalInput")
    biascol = nc.dram_tensor("biascol", [P, 3], F32, kind="ExternalInput")
    id_f32 = nc.dram_tensor("id_f32", [P, P], F32, kind="ExternalInput")
    id_bf = nc.dram_tensor("id_bf", [P, P], BF16, kind="ExternalInput")
    iota_d = nc.dram_tensor("iota_bf", [P, 8 * P], BF16, kind="ExternalInput")
    out = nc.dram_tensor("out", [P, 2 * NS * cfg.GPC], F32, kind="ExternalOutput")

    rg = [list(range(cfg.NCORES))]

    with tile.TileContext(nc) as tc:
        with (
            tc.tile_pool(name="const", bufs=1) as constp,
            tc.tile_pool(name="msg", bufs=4) as msgp,
            tc.tile_pool(name="msgh", bufs=4) as msghp,
            tc.tile_pool(name="otile", bufs=3) as otp,
            tc.tile_pool(name="oth", bufs=3) as othp,
            tc.tile_pool(name="selfr", bufs=3) as selfp,
            tc.tile_pool(name="work", bufs=2) as workp,
            tc.tile_pool(name="psig", bufs=2) as psigp,
            tc.tile_pool(name="pool", bufs=1) as poolp,
            tc.tile_pool(name="gps", bufs=4, space="PSUM") as gpsp,
            tc.tile_pool(name="t1ps", bufs=1, space="PSUM") as t1psp,
            tc.tile_pool(name="psips", bufs=2, space="PSUM") as psipsp,
            tc.tile_pool(name="t2ps", bufs=1, space="PSUM") as t2psp,
            tc.tile_pool(name="dram", bufs=1, space="DRAM") as dramp,
        ):
            # ---- constants into SBUF
            idx_sb = constp.tile([P, NCALLS * 64], I16)
            nc.sync.dma_start(out=idx_sb[:], in_=gidx_d[:])
            dstl_sb = constp.tile([P, NCALLS * 8], BF16)
            nc.sync.dma_start(out=dstl_sb[:], in_=dstl_d[:])
            wzt = constp.tile([P, 12 * P], BF16, tag="wzt")
            nc.sync.dma_start(
                out=wzt[:].rearrange("p (i m) -> p i m", i=12),
                in_=wz_d.ap().rearrange("i p m -> p i m"))
            bct = constp.tile([P, 3], F32)
            nc.sync.dma_start(out=bct[:], in_=biascol[:])
            idf = constp.tile([P, P], F32)
            nc.sync.dma_start(out=idf[:], in_=id_f32[:])
            idb = constp.tile([P, P], BF16)
            nc.sync.dma_start(out=idb[:], in_=id_bf[:])
            iota_sb = constp.tile([P, 8 * P], BF16)
            nc.sync.dma_start(out=iota_sb[:], in_=iota_d[:])
            dinv_sb = constp.tile([P, cfg.NBLK], F32)
            nc.sync.dma_start(out=dinv_sb[:], in_=dinv_d[:])

            # ---- pool accumulators
            lmax = poolp.tile([P, NS * cfg.GPC], F32, tag="lmax")
            lsum = poolp.tile([P, NS * cfg.GPC], F32, tag="lsum")
            fmax = poolp.tile([P, NS * cfg.GPC], F32, tag="fmax")
            fsum = poolp.tile([P, NS * cfg.GPC], F32, tag="fsum")
            for _t in (lmax, lsum, fmax, fsum):
                nc.vector.memset(_t[:], 0.0)

            # ---- DRAM intermediates
            SEGB = cfg.SEGB
            NSEG = len(SEGB) - 1
            h_mine, h_full = [], []
            for i in range(2):
                hm = dramp.tile([cfg.NPC, CH], BF16, tag=f"hm{i}")
                h_mine.append(hm)
                segs = []
                for s in range(NSEG):
                    nb = SEGB[s + 1] - SEGB[s]
                    hfs = dramp.tile([cfg.NCORES * nb * P, CH], BF16,
                                     tag=f"hf{i}s{s}", addr_space="Shared")
                    segs.append(hfs)
                h_full.append(segs)

            def layer(li):
                ch_in = CH1 if li == 0 else CH
                ns_in = NS1 if li == 0 else NS
                if li > 0:
                    src_seg = [h[:, :] for h in h_full[li - 1]]

                calls_of_group = {}
                for ci, (h, gi, pos, k) in enumerate(calls):
                    calls_of_group.setdefault(gi, []).append((ci, h, pos, k))

                for gi, blks in enumerate(groups):
                    gtiles = {}
                    for ci, h, pos, k in calls_of_group.get(gi, []):
                        ni = k * P
                        g = (msgp if h < 2 else msghp).tile(
                            [P, 8 * ch_in], BF16, tag=f"m{h}")
                        if li == 0:
                            nc.sync.dma_start(out=g[:, :k * ch_in],
                                              in_=xe_d[ci, :, :k * ch_in])
                        else:
                            nc.gpsimd.dma_gather(
                                out_ap=g[:, :k * ch_in].rearrange(
                                    "p (c e) -> p c e", e=ch_in),
                                in_ap=src_seg[h],
                                idxs_ap=idx_sb[:, ci * 64: ci * 64 + max(ni // 16, 1)],
                                num_idxs=ni,
                                num_idxs_reg=ni,
                                elem_size=ch_in,
                                queue_num=ci % 4,
                            )
                        ot = (otp if h < 2 else othp).tile([P, 8 * P], BF16,
                                                           tag=f"oo{h}")
                        nc.vector.tensor_tensor(
                            out=ot[:, :k * P].rearrange("p (c d) -> p c d", d=P),
                            in0=dstl_sb[:, ci * 8: ci * 8 + k
                                        ].to_broadcast([P, k, P]),
                            in1=iota_sb[:, :k * P].rearrange(
                                "p (c d) -> p c d", d=P),
                            op=mybir.AluOpType.is_equal)
                        gtiles[ci] = (g, ot)

                    psi_grp = psigp.tile([P, NS * len(blks) * P], F32, tag="psig")
                    for bi, b in enumerate(blks):
                        # self-loop rhs: own rows (X' for L0, H' for L1/2)
                        selfr = selfp.tile([P, ch_in], BF16, tag="selfr")
                        if li == 0:
                            nc.sync.dma_start(out=selfr[:],
                                              in_=xself_d[b * P:(b + 1) * P, :])
                        else:
                            nc.sync.dma_start(
                                out=selfr[:],
                                in_=h_mine[li - 1][b * P:(b + 1) * P, :])
                        # chunks of this block
                        mychunks = []
                        for h in range(NSEG):
                            own = owners[(h, gi)]
                            for cpos, (obi, k) in enumerate(own):
                                if obi == bi:
                                    mychunks.append(chunk_map[(h, gi, cpos)])
                        gps = gpsp.tile([P, ch_in], F32, tag="gps")
                        nc.tensor.matmul(gps[:], lhsT=idb[:], rhs=selfr[:],
                                         start=True, stop=(len(mychunks) == 0))
                        for mi, (ci, j) in enumerate(mychunks):
                            g, ot = gtiles[ci]
                            nc.tensor.matmul(
                                gps[:],
                                lhsT=ot[:, j * P:(j + 1) * P],
                                rhs=g[:, j * ch_in:(j + 1) * ch_in],
                                start=False, stop=(mi == len(mychunks) - 1),
                            )
                        # ---- epilogue for block b
                        gbf = workp.tile([P, ch_in], F32, tag="gbf")
                        nc.scalar.activation(gbf[:], gps[:],
                                             mybir.ActivationFunctionType.Copy,
                                             scale=dinv_sb[:, b:b + 1])
                        t1 = t1psp.tile([P, ns_in * P], F32, tag="t1")
                        for s in range(ns_in):
                            nc.tensor.transpose(
                                t1[:, s * P:(s + 1) * P],
                                gbf[:, s * P:(s + 1) * P], idf[:])
                        gt = workp.tile([P, ns_in * P], BF16, tag="gt")
                        nc.scalar.activation(gt[:], t1[:],
                                             mybir.ActivationFunctionType.Copy)
                        psi_ps = psipsp.tile([P, NS * P], F32, tag="psip")
                        kdim = CIN if li == 0 else COUT
                        nq = P // kdim
                        for t_ in range(T):
                            s_out = t_ // 2
                            q = t_ % nq
                            s_in = t_ // nq
                            nc.tensor.matmul(
                                psi_ps[:, s_out * P:(s_out + 1) * P],
                                lhsT=wzt[:, (li * 4 + q) * P:(li * 4 + q + 1) * P],
                                rhs=gt[:, s_in * P:(s_in + 1) * P],
                                start=(t_ % 2 == 0), stop=(t_ % 2 == 1),
                            )
                        gwk = len(blks) * P
                        dst_view = psi_grp[:].rearrange(
                            "p (s n) -> p s n", n=gwk)[:, :, bi * P:(bi + 1) * P]
                        nc.scalar.activation(
                            dst_view,
                            psi_ps[:].rearrange("p (s n) -> p s n", s=NS),
                            mybir.ActivationFunctionType.Relu,
                            bias=bct[:, li:li + 1],
                        )
                        if li < 2:
                            t2 = t2psp.tile([P, NS * P], F32, tag="t2")
                            for s in range(NS):
                                nc.tensor.transpose(
                                    t2[:, s * P:(s + 1) * P],
                                    psi_grp[:, s * len(blks) * P + bi * P:
                                            s * len(blks) * P + (bi + 1) * P],
                                    idf[:])
                            hbf = workp.tile([P, CH], BF16, tag="hbf")
                            nc.vector.tensor_scalar_mul(hbf[:], t2[:],
                                                        dinv_sb[:, b:b + 1])
                            nc.sync.dma_start(
                                out=h_mine[li][b * P:(b + 1) * P, :], in_=hbf[:])

                    # ---- pooling for this group
                    gw = len(blks) * P
                    for s in range(NS):
                        base = s * gw
                        for (n0, n1, gl, ft) in pool_pieces[gi]:
                            seg = psi_grp[:, base + n0: base + n1]
                            if ft:
                                nc.vector.reduce_max(
                                    out=lmax[:, s * cfg.GPC + gl: s * cfg.GPC + gl + 1],
                                    in_=seg, axis=mybir.AxisListType.X)
                                nc.vector.reduce_sum(
                                    out=lsum[:, s * cfg.GPC + gl: s * cfg.GPC + gl + 1],
                                    in_=seg, axis=mybir.AxisListType.X)
                            else:
                                tm = workp.tile([P, 2], F32, tag="ptmp")
                                nc.vector.reduce_max(out=tm[:, 0:1], in_=seg,
                                                     axis=mybir.AxisListType.X)
                                nc.vector.reduce_sum(out=tm[:, 1:2], in_=seg,
                                                     axis=mybir.AxisListType.X)
                                nc.vector.tensor_tensor(
                                    out=lmax[:, s * cfg.GPC + gl: s * cfg.GPC + gl + 1],
                                    in0=lmax[:, s * cfg.GPC + gl: s * cfg.GPC + gl + 1],
                                    in1=tm[:, 0:1], op=mybir.AluOpType.max)
                                nc.vector.tensor_add(
                                    out=lsum[:, s * cfg.GPC + gl: s * cfg.GPC + gl + 1],
                                    in0=lsum[:, s * cfg.GPC + gl: s * cfg.GPC + gl + 1],
                                    in1=tm[:, 1:2])

                    # progressive allgathers: each segment fires as soon as
                    # its blocks are done; the last segment is tiny, so the
                    # layer-boundary stall is small.
                    if li < 2 and int(os.environ.get("GCN_LAYERS", "3")) > li + 1:
                        for s in range(NSEG - 1):
                            if gi == (SEGB[s + 1] - 1) // cfg.GRP:
                                nc.gpsimd.collective_compute(
                                    "AllGather", mybir.AluOpType.bypass,
                                    replica_groups=rg,
                                    ins=[h_mine[li][SEGB[s] * P:
                                                    SEGB[s + 1] * P, :]],
                                    outs=[h_full[li][s][:, :]],
                                )

                # ---- layer end: accumulate pools
                if li == 0:
                    nc.vector.tensor_copy(out=fmax[:], in_=lmax[:])
                    nc.vector.tensor_copy(out=fsum[:], in_=lsum[:])
                else:
                    nc.vector.tensor_add(out=fmax[:], in0=fmax[:], in1=lmax[:])
                    nc.vector.tensor_add(out=fsum[:], in0=fsum[:], in1=lsum[:])

                if li < 2 and int(os.environ.get("GCN_LAYERS", "3")) > li + 1:
                    s = NSEG - 1
                    nc.gpsimd.collective_compute(
                        "AllGather", mybir.AluOpType.bypass,
                        replica_groups=rg,
                        ins=[h_mine[li][SEGB[s] * P:SEGB[s + 1] * P, :]],
                        outs=[h_full[li][s][:, :]],
                    )

            for _li in range(int(os.environ.get("GCN_LAYERS", "3"))):
                layer(_li)

            nc.vector.tensor_scalar_mul(fsum[:], fsum[:],
                                        float(np.float32(1.0 / cfg.GRAPH)))
            osb = workp.tile([P, 2 * NS * cfg.GPC], F32, tag="osb")
            nc.vector.tensor_copy(out=osb[:, :NS * cfg.GPC], in_=fmax[:])
            nc.vector.tensor_copy(out=osb[:, NS * cfg.GPC:], in_=fsum[:])
            nc.sync.dma_start(out=out[:], in_=osb[:])

    nc.compile()
    return nc


def unshard(cfg, results):
    """[NCORES][128, 2*NS*GPC] -> [B, 2*COUT, T] float32."""
    B, T, COUT, NS, GPC = cfg.B, cfg.T, cfg.COUT, cfg.NS, cfg.GPC
    out = np.zeros((B, 2 * COUT, T), np.float32)
    for c in range(cfg.NCORES):
        V = results[c]["out"]
        for gl in range(cfg.gpc[c]):
            g = cfg.goff[c] + gl
            for s in range(NS):
                for half in range(2):
                    t_ = 2 * s + half
                    co = np.arange(COUT)
                    pp = half * COUT + co
                    out[g, co, t_] = V[pp, s * GPC + gl]
                    out[g, COUT + co, t_] = V[pp, NS * GPC + s * GPC + gl]
    return out


_CACHE = {}


def kernel(**inputs):
    cfg = Cfg()
    common, per_core, meta = preprocess(
        cfg, inputs["x"], inputs["edge_index"], inputs["batch"],
        inputs["W1"], inputs["b1"], inputs["W2"], inputs["b2"],
        inputs["W3"], inputs["b3"])
    key = (tuple(meta["Kbh"].flatten().tolist()), meta["NCALLS"])
    if key not in _CACHE:
        _CACHE[key] = build(cfg, meta)
    nc = _CACHE[key]
    in_maps = []
    for c in range(cfg.NCORES):
        m = dict(common)
        m.update(per_core[c])
        in_maps.append(m)
    res = run_bass_kernel_spmd(nc, in_maps, list(range(cfg.NCORES)))
    return unshard(cfg, res.results)


# revision 26
# speedup vs baseline: 1.1429x; 1.0225x over previous
"""Trainium2 Bass kernel for nn_GCNLayer (3-layer GCN + max/mean pooling, T temporal slices).

Self-contained: hardcodes the problem shapes (N=50000, E=800000, B=250, T=8,
CIN=32, COUT=64) and distributes over 8 NeuronCores by graph/dst-node range.

v2 design (vs v1 baseline):
  - Layer-0 messages are pre-gathered on the HOST into edge-chunk order (xe)
    and streamed with plain sequential DMA - no gpsimd descriptor generation.
  - The one-hot scatter matrices are generated ON DEVICE per chunk via
    is_equal(dstl, iota) on the vector engine (0/1 entries), instead of
    loading ~34MB/layer of precomputed one-hots from HBM.
  - The symmetric norm dinv[src]*dinv[dst] is factored: rows stored in DRAM
    carry dinv[src] (host-baked for X, fused into the epilogue copy for H),
    and the aggregated block result is scaled by dinv[dst] (fused into the
    PSUM->SBUF copy).
  - Self-loops are not gathered: an identity matmul over the core's own
    contiguous rows adds them.
  - Per-(block,half) chunk counts (max over cores) instead of a uniform
    KL/KH, cutting padded descriptor count.
  - dma_gather calls round-robin over 4 SWDGE queues.
"""

import os
import numpy as np
import ml_dtypes

import concourse.bass as bass
import concourse.mybir as mybir
from concourse import bacc, tile
from concourse.bass_utils import run_bass_kernel_spmd

F32 = mybir.dt.float32
BF16 = mybir.dt.bfloat16
I16 = mybir.dt.int16
P = 128


class Cfg:
    def __init__(self, N=50000, E=800000, B=250, T=8, CIN=32, COUT=64,
                 NCORES=8, GRAPH=200):
        self.N, self.E, self.B, self.T = N, E, B, T
        self.CIN, self.COUT, self.NCORES, self.GRAPH = CIN, COUT, NCORES, GRAPH
        base, rem = divmod(B, NCORES)
        self.gpc = [base + (1 if c < rem else 0) for c in range(NCORES)]
        self.GPC = max(self.gpc)                      # uniform per-core graph slots
        self.NPC = self.GPC * GRAPH                   # padded nodes per core
        assert self.NPC % P == 0
        self.NBLK = self.NPC // P                     # dst blocks per core
        self.NPAD = self.NPC * NCORES                 # padded global node count
        self.HALF = self.NPAD // 2                    # gather index split point
        assert self.HALF <= 32767 + 1
        self.CH1 = CIN * T                            # layer-1 feature row
        self.CH = COUT * T                            # layer-2/3 feature row
        assert self.CH % P == 0
        self.NS = self.CH // P                        # psi partition tiles (t-pairs)
        self.GRP = 4 if self.NBLK >= 4 else self.NBLK
        self.SEGB = [0, 25, self.NBLK]    # segment block boundaries
        self.goff = np.concatenate([[0], np.cumsum(self.gpc)]).astype(np.int64)


def _wrap_idx_into(idx, ci, vals):
    """Write 128 int values for call ci chunk-position handled by caller.
    vals: flat positions i0..i0+127 handled by caller; here vals is (i0, arr)."""
    i0, arr = vals
    ii = i0 + np.arange(len(arr))
    idx[ii % 16, ci * 64 + ii // 16] = arr.astype(np.int16)


def preprocess(cfg, x, edge_index, batch, W1, b1, W2, b2, W3, b3):
    """Build all per-core device inputs. Returns (common, per_core_inputs, meta)."""
    N, E, T, CIN, COUT = cfg.N, cfg.E, cfg.T, cfg.CIN, cfg.COUT
    src = np.asarray(edge_index[0], np.int64)
    dst = np.asarray(edge_index[1], np.int64)

    # degrees incl self-loops, matching the reference
    deg = np.bincount(dst, minlength=N).astype(np.float32) + 1.0
    dinv = (1.0 / np.sqrt(deg)).astype(np.float32)

    batch = np.asarray(batch, np.int64)
    g2c = np.zeros(cfg.B, np.int64)
    for c in range(cfg.NCORES):
        g2c[cfg.goff[c]:cfg.goff[c + 1]] = c
    node_core = g2c[batch]
    first_node_of_core = np.array(
        [cfg.goff[c] * cfg.GRAPH for c in range(cfg.NCORES)], np.int64)
    local_n = np.arange(N) - first_node_of_core[node_core]
    CHK = cfg.NPC // 2

    # X' = dinv * X, row-major [N, T*CIN] (t-major rows)
    xm = np.moveaxis(np.asarray(x, np.float32), 2, 1).reshape(N, T * CIN)
    xs = xm * dinv[:, None]

    # 4 node segments by local block range; each gets its own rank-major
    # shared gather tensor and its own early AllGather.
    SEGB = np.asarray(cfg.SEGB, np.int64)
    NSEG = len(SEGB) - 1
    src_blk = local_n[src] // P
    src_seg = np.searchsorted(SEGB, src_blk, side="right") - 1
    seg_nb = np.diff(SEGB)
    srcp = (node_core[src] * (seg_nb[src_seg] * P)
            + (local_n[src] - SEGB[src_seg] * P))
    dstc = node_core[dst]
    dstl = local_n[dst]

    # ---- per-core edge bucketing (NO self loops)
    per_core = []
    cnt = np.zeros((cfg.NCORES, cfg.NBLK, NSEG), np.int64)
    for c in range(cfg.NCORES):
        m = dstc == c
        es, ed, so, sg = srcp[m], dstl[m], src[m], src_seg[m]
        blk = ed // P
        order = np.lexsort((es, sg, blk))
        es, ed, so, blk, sg = es[order], ed[order], so[order], blk[order], sg[order]
        per_core.append((es, ed, so, blk, sg))
        for b in range(cfg.NBLK):
            mb = blk == b
            for s in range(NSEG):
                cnt[c, b, s] = int(((sg == s) & mb).sum())

    # per-(block,seg) chunk count = max over cores
    K = -(-cnt // P)            # ceil
    Kbh = K.max(axis=0)         # [NBLK, NSEG]

    groups = []
    b0 = 0
    while b0 < cfg.NBLK:
        groups.append(list(range(b0, min(b0 + cfg.GRP, cfg.NBLK))))
        b0 += cfg.GRP

    # call table: (half, group, pos, k) with chunk owner lists
    calls = []
    chunk_map = {}              # (h, gi, chunk_pos_in_group) -> (ci, j)
    owners = {}                 # (h, gi) -> list of (bi, k_index) per chunk pos
    for h in range(NSEG):
        for gi, blks in enumerate(groups):
            own = []
            for bi, b in enumerate(blks):
                for k in range(int(Kbh[b, h])):
                    own.append((bi, k))
            owners[(h, gi)] = own
            pos = 0
            while pos < len(own):
                k = min(8, len(own) - pos)
                ci = len(calls)
                calls.append((h, gi, pos, k))
                for j in range(k):
                    chunk_map[(h, gi, pos + j)] = (ci, j)
                pos += k
            if not own:
                pass
    NCALLS = len(calls)

    # ---- per-core tensors
    per_core_inputs = []
    for c in range(cfg.NCORES):
        es, ed, so, blk, sg = per_core[c]
        gidx = np.zeros((16, NCALLS * 64), np.int16)
        dstl_t = np.full((P, NCALLS * 8), 200.0, np.float32)
        xe = np.zeros((NCALLS, P, 8 * cfg.CH1), ml_dtypes.bfloat16)
        # slice edges per (b, h)
        starts = {}
        i0 = 0
        for b in range(cfg.NBLK):
            for h in range(NSEG):
                n_e = int(cnt[c, b, h])
                starts[(b, h)] = i0
                i0 += n_e
        for h in range(NSEG):
            for gi, blks in enumerate(groups):
                own = owners[(h, gi)]
                for cpos, (bi, k) in enumerate(own):
                    b = blks[bi]
                    ci, j = chunk_map[(h, gi, cpos)]
                    e0 = starts[(b, h)] + k * P
                    e1 = min(starts[(b, h)] + int(cnt[c, b, h]), e0 + P)
                    ne = max(0, e1 - e0)
                    vals = np.zeros(P, np.int64)
                    if ne:
                        vals[:ne] = es[e0:e1]
                        dstl_t[:ne, ci * 8 + j] = (ed[e0:e1] - b * P).astype(np.float32)
                        xe[ci, :ne, j * cfg.CH1:(j + 1) * cfg.CH1] = \
                            xs[so[e0:e1]].astype(ml_dtypes.bfloat16)
                    ii = j * P + np.arange(P)
                    gidx[ii % 16, ci * 64 + ii // 16] = vals.astype(np.int16)
        gidx = np.tile(gidx, (8, 1))

        # core's own X' rows in local order (self-loop rhs), pad rows zero
        nreal = cfg.gpc[c] * cfg.GRAPH
        xself = np.zeros((cfg.NPC, cfg.CH1), np.float32)
        xself[:nreal] = xs[first_node_of_core[c]:first_node_of_core[c] + nreal]

        # dinv per local node as [128, NBLK]
        dv = np.zeros(cfg.NPC, np.float32)
        dv[:nreal] = dinv[first_node_of_core[c]:first_node_of_core[c] + nreal]
        dinvcol = dv.reshape(cfg.NBLK, P).T.copy()

        per_core_inputs.append({
            "gidx": gidx,
            "dstl": dstl_t.astype(ml_dtypes.bfloat16),
            "xe": xe,
            "xself": xself.astype(ml_dtypes.bfloat16),
            "dinvcol": dinvcol,
        })

    # weights: zero-padded [128,128] lhsT variants
    wz = np.zeros((12, P, P), np.float32)
    for li, W in enumerate((W1, W2, W3)):
        W = np.asarray(W, np.float32)
        kdim = W.shape[0]
        nq = P // kdim
        for q in range(nq):
            hh = q % 2
            wz[li * 4 + q, q * kdim:(q + 1) * kdim, hh * COUT:(hh + 1) * COUT] = W

    bias_col = np.zeros((P, 3), np.float32)
    for i, b in enumerate((b1, b2, b3)):
        bias_col[:, i] = np.tile(np.asarray(b, np.float32), P // COUT)

    ident = np.eye(P, dtype=np.float32)
    iota = np.tile(np.arange(P, dtype=np.float32)[None, :], (P, 8))

    # pooling piece table per group
    pool_pieces = []
    seen = set()
    for gi, blks in enumerate(groups):
        n0g, n1g = blks[0] * P, (blks[-1] + 1) * P
        pieces = []
        n = n0g
        while n < n1g:
            gl = n // cfg.GRAPH
            nend = min((gl + 1) * cfg.GRAPH, n1g)
            ft = gl not in seen
            seen.add(gl)
            pieces.append((n - n0g, nend - n0g, gl, ft))
            n = nend
        pool_pieces.append(pieces)

    common = {
        "wz": wz.astype(ml_dtypes.bfloat16),
        "biascol": bias_col,
        "id_f32": ident,
        "id_bf": ident.astype(ml_dtypes.bfloat16),
        "iota_bf": iota.astype(ml_dtypes.bfloat16),
    }
    meta = dict(Kbh=Kbh, calls=calls, chunk_map=chunk_map, owners=owners,
                groups=groups, pool_pieces=pool_pieces, NCALLS=NCALLS)
    return common, per_core_inputs, meta


def build(cfg, meta):
    calls, chunk_map, owners = meta["calls"], meta["chunk_map"], meta["owners"]
    groups, pool_pieces, NCALLS = meta["groups"], meta["pool_pieces"], meta["NCALLS"]
    NS, CH, CH1, T, COUT, CIN = cfg.NS, cfg.CH, cfg.CH1, cfg.T, cfg.COUT, cfg.CIN
    NS1 = max(CH1 // P, 1)

    nc = bacc.Bacc("TRN2", target_bir_lowering=False, debug=False,
                   num_devices=cfg.NCORES, num_swdge_queues=4)

    gidx_d = nc.dram_tensor("gidx", [P, NCALLS * 64], I16, kind="ExternalInput")
    dstl_d = nc.dram_tensor("dstl", [P, NCALLS * 8], BF16, kind="ExternalInput")
    xe_d = nc.dram_tensor("xe", [NCALLS, P, 8 * CH1], BF16, kind="ExternalInput")
    xself_d = nc.dram_tensor("xself", [cfg.NPC, CH1], BF16, kind="ExternalInput")
    dinv_d = nc.dram_tensor("dinvcol", [P, cfg.NBLK], F32, kind="ExternalInput")
    wz_d = nc.dram_tensor("wz", [12, P, P], BF16, kind="ExternalInput")
    biascol = nc.dram_tensor("biascol", [P, 3], F32, kind="ExternalInput")
    id_f32 = nc.dram_tensor("id_f32", [P, P], F32, kind="ExternalInput")
    id_bf = nc.dram_tensor("id_bf", [P, P], BF16, kind="ExternalInput")
    iota_d = nc.dram_tensor("iota_bf", [P, 8 * P], BF16, kind="ExternalInput")
    out = nc.dram_tensor("out", [P, 2 * NS * cfg.GPC], F32, kind="ExternalOutput")

    rg = [list(range(cfg.NCORES))]

    with tile.TileContext(nc) as tc:
        with (
            tc.tile_pool(name="const", bufs=1) as constp,
            tc.tile_pool(name="msg", bufs=7) as msgp,
            tc.tile_pool(name="msgh", bufs=4) as msghp,
            tc.tile_pool(name="otile", bufs=7) as otp,
            tc.tile_pool(name="oth", bufs=3) as othp,
            tc.tile_pool(name="selfr", bufs=3) as selfp,
            tc.tile_pool(name="work", bufs=2) as workp,
            tc.tile_pool(name="psig", bufs=2) as psigp,
            tc.tile_pool(name="pool", bufs=1) as poolp,
            tc.tile_pool(name="gps", bufs=4, space="PSUM") as gpsp,
            tc.tile_pool(name="t1ps", bufs=1, space="PSUM") as t1psp,
            tc.tile_pool(name="psips", bufs=2, space="PSUM") as psipsp,
            tc.tile_pool(name="t2ps", bufs=1, space="PSUM") as t2psp,
            tc.tile_pool(name="dram", bufs=1, space="DRAM") as dramp,
        ):
            # ---- constants into SBUF
            idx_sb = constp.tile([P, NCALLS * 64], I16)
            nc.sync.dma_start(out=idx_sb[:], in_=gidx_d[:])
            dstl_sb = constp.tile([P, NCALLS * 8], BF16)
            nc.sync.dma_start(out=dstl_sb[:], in_=dstl_d[:])
            wzt = constp.tile([P, 12 * P], BF16, tag="wzt")
            nc.sync.dma_start(
                out=wzt[:].rearrange("p (i m) -> p i m", i=12),
                in_=wz_d.ap().rearrange("i p m -> p i m"))
            bct = constp.tile([P, 3], F32)
            nc.sync.dma_start(out=bct[:], in_=biascol[:])
            idf = constp.tile([P, P], F32)
            nc.sync.dma_start(out=idf[:], in_=id_f32[:])
            idb = constp.tile([P, P], BF16)
            nc.sync.dma_start(out=idb[:], in_=id_bf[:])
            iota_sb = constp.tile([P, 8 * P], BF16)
            nc.sync.dma_start(out=iota_sb[:], in_=iota_d[:])
            dinv_sb = constp.tile([P, cfg.NBLK], F32)
            nc.sync.dma_start(out=dinv_sb[:], in_=dinv_d[:])

            # ---- pool accumulators
            lmax = poolp.tile([P, NS * cfg.GPC], F32, tag="lmax")
            lsum = poolp.tile([P, NS * cfg.GPC], F32, tag="lsum")
            fmax = poolp.tile([P, NS * cfg.GPC], F32, tag="fmax")
            fsum = poolp.tile([P, NS * cfg.GPC], F32, tag="fsum")
            for _t in (lmax, lsum, fmax, fsum):
                nc.vector.memset(_t[:], 0.0)

            # ---- DRAM intermediates
            SEGB = cfg.SEGB
            NSEG = len(SEGB) - 1
            h_mine, h_full = [], []
            for i in range(2):
                hm = dramp.tile([cfg.NPC, CH], BF16, tag=f"hm{i}")
                h_mine.append(hm)
                segs = []
                for s in range(NSEG):
                    nb = SEGB[s + 1] - SEGB[s]
                    hfs = dramp.tile([cfg.NCORES * nb * P, CH], BF16,
                                     tag=f"hf{i}s{s}", addr_space="Shared")
                    segs.append(hfs)
                h_full.append(segs)

            def emit_call(li, ci, h, k, gtiles):
                ch_in = CH1 if li == 0 else CH
                ni = k * P
                g = (msgp if h < 2 else msghp).tile(
                    [P, 8 * ch_in], BF16, tag=f"m{h}")
                if li == 0:
                    nc.sync.dma_start(out=g[:, :k * ch_in],
                                      in_=xe_d[ci, :, :k * ch_in])
                else:
                    nc.gpsimd.dma_gather(
                        out_ap=g[:, :k * ch_in].rearrange(
                            "p (c e) -> p c e", e=ch_in),
                        in_ap=h_full[li - 1][h][:, :],
                        idxs_ap=idx_sb[:, ci * 64: ci * 64 + max(ni // 16, 1)],
                        num_idxs=ni,
                        num_idxs_reg=ni,
                        elem_size=ch_in,
                        queue_num=ci % 4,
                    )
                ot = (otp if h < 2 else othp).tile([P, 8 * P], BF16,
                                                   tag=f"oo{h}")
                nc.vector.tensor_tensor(
                    out=ot[:, :k * P].rearrange("p (c d) -> p c d", d=P),
                    in0=dstl_sb[:, ci * 8: ci * 8 + k
                                ].to_broadcast([P, k, P]),
                    in1=iota_sb[:, :k * P].rearrange(
                        "p (c d) -> p c d", d=P),
                    op=mybir.AluOpType.is_equal)
                gtiles[ci] = (g, ot)

            calls_of_group = {}
            for ci, (h, gi, pos, k) in enumerate(calls):
                calls_of_group.setdefault(gi, []).append((ci, h, pos, k))

            def layer(li, pre_gtiles):
                ch_in = CH1 if li == 0 else CH
                ns_in = NS1 if li == 0 else NS

                for gi, blks in enumerate(groups):
                    gtiles = {}
                    for ci, h, pos, k in calls_of_group.get(gi, []):
                        if ci in pre_gtiles:
                            gtiles[ci] = pre_gtiles[ci]
                            continue
                        emit_call(li, ci, h, k, gtiles)

                    psi_grp = psigp.tile([P, NS * len(blks) * P], F32, tag="psig")
                    for bi, b in enumerate(blks):
                        # self-loop rhs: own rows (X' for L0, H' for L1/2)
                        selfr = selfp.tile([P, ch_in], BF16, tag="selfr")
                        if li == 0:
                            nc.sync.dma_start(out=selfr[:],
                                              in_=xself_d[b * P:(b + 1) * P, :])
                        else:
                            nc.sync.dma_start(
                                out=selfr[:],
                                in_=h_mine[li - 1][b * P:(b + 1) * P, :])
                        # chunks of this block
                        mychunks = []
                        for h in range(NSEG):
                            own = owners[(h, gi)]
                            for cpos, (obi, k) in enumerate(own):
                                if obi == bi:
                                    mychunks.append(chunk_map[(h, gi, cpos)])
                        gps = gpsp.tile([P, ch_in], F32, tag="gps")
                        nc.tensor.matmul(gps[:], lhsT=idb[:], rhs=selfr[:],
                                         start=True, stop=(len(mychunks) == 0))
                        for mi, (ci, j) in enumerate(mychunks):
                            g, ot = gtiles[ci]
                            nc.tensor.matmul(
                                gps[:],
                                lhsT=ot[:, j * P:(j + 1) * P],
                                rhs=g[:, j * ch_in:(j + 1) * ch_in],
                                start=False, stop=(mi == len(mychunks) - 1),
                            )
                        # ---- epilogue for block b
                        gbf = workp.tile([P, ch_in], F32, tag="gbf")
                        nc.scalar.activation(gbf[:], gps[:],
                                             mybir.ActivationFunctionType.Copy,
                                             scale=dinv_sb[:, b:b + 1])
                        t1 = t1psp.tile([P, ns_in * P], F32, tag="t1")
                        for s in range(ns_in):
                            nc.tensor.transpose(
                                t1[:, s * P:(s + 1) * P],
                                gbf[:, s * P:(s + 1) * P], idf[:])
                        gt = workp.tile([P, ns_in * P], BF16, tag="gt")
                        nc.scalar.activation(gt[:], t1[:],
                                             mybir.ActivationFunctionType.Copy)
                        psi_ps = psipsp.tile([P, NS * P], F32, tag="psip")
                        kdim = CIN if li == 0 else COUT
                        nq = P // kdim
                        for t_ in range(T):
                            s_out = t_ // 2
                            q = t_ % nq
                            s_in = t_ // nq
                            nc.tensor.matmul(
                                psi_ps[:, s_out * P:(s_out + 1) * P],
                                lhsT=wzt[:, (li * 4 + q) * P:(li * 4 + q + 1) * P],
                                rhs=gt[:, s_in * P:(s_in + 1) * P],
                                start=(t_ % 2 == 0), stop=(t_ % 2 == 1),
                            )
                        gwk = len(blks) * P
                        dst_view = psi_grp[:].rearrange(
                            "p (s n) -> p s n", n=gwk)[:, :, bi * P:(bi + 1) * P]
                        nc.scalar.activation(
                            dst_view,
                            psi_ps[:].rearrange("p (s n) -> p s n", s=NS),
                            mybir.ActivationFunctionType.Relu,
                            bias=bct[:, li:li + 1],
                        )
                        if li < 2:
                            t2 = t2psp.tile([P, NS * P], F32, tag="t2")
                            for s in range(NS):
                                nc.tensor.transpose(
                                    t2[:, s * P:(s + 1) * P],
                                    psi_grp[:, s * len(blks) * P + bi * P:
                                            s * len(blks) * P + (bi + 1) * P],
                                    idf[:])
                            hbf = workp.tile([P, CH], BF16, tag="hbf")
                            nc.vector.tensor_scalar_mul(hbf[:], t2[:],
                                                        dinv_sb[:, b:b + 1])
                            nc.sync.dma_start(
                                out=h_mine[li][b * P:(b + 1) * P, :], in_=hbf[:])

                    # ---- pooling for this group
                    gw = len(blks) * P
                    for s in range(NS):
                        base = s * gw
                        for (n0, n1, gl, ft) in pool_pieces[gi]:
                            seg = psi_grp[:, base + n0: base + n1]
                            if ft:
                                nc.vector.reduce_max(
                                    out=lmax[:, s * cfg.GPC + gl: s * cfg.GPC + gl + 1],
                                    in_=seg, axis=mybir.AxisListType.X)
                                nc.vector.reduce_sum(
                                    out=lsum[:, s * cfg.GPC + gl: s * cfg.GPC + gl + 1],
                                    in_=seg, axis=mybir.AxisListType.X)
                            else:
                                tm = workp.tile([P, 2], F32, tag="ptmp")
                                nc.vector.reduce_max(out=tm[:, 0:1], in_=seg,
                                                     axis=mybir.AxisListType.X)
                                nc.vector.reduce_sum(out=tm[:, 1:2], in_=seg,
                                                     axis=mybir.AxisListType.X)
                                nc.vector.tensor_tensor(
                                    out=lmax[:, s * cfg.GPC + gl: s * cfg.GPC + gl + 1],
                                    in0=lmax[:, s * cfg.GPC + gl: s * cfg.GPC + gl + 1],
                                    in1=tm[:, 0:1], op=mybir.AluOpType.max)
                                nc.vector.tensor_add(
                                    out=lsum[:, s * cfg.GPC + gl: s * cfg.GPC + gl + 1],
                                    in0=lsum[:, s * cfg.GPC + gl: s * cfg.GPC + gl + 1],
                                    in1=tm[:, 1:2])

                    # progressive allgathers: each segment fires as soon as
                    # its blocks are done; the last segment is tiny, so the
                    # layer-boundary stall is small.
                    if li < 2 and int(os.environ.get("GCN_LAYERS", "3")) > li + 1:
                        for s in range(NSEG - 1):
                            if gi == (SEGB[s + 1] - 1) // cfg.GRP:
                                nc.gpsimd.collective_compute(
                                    "AllGather", mybir.AluOpType.bypass,
                                    replica_groups=rg,
                                    ins=[h_mine[li][SEGB[s] * P:
                                                    SEGB[s + 1] * P, :]],
                                    outs=[h_full[li][s][:, :]],
                                )

                # ---- layer end: accumulate pools
                if li == 0:
                    nc.vector.tensor_copy(out=fmax[:], in_=lmax[:])
                    nc.vector.tensor_copy(out=fsum[:], in_=lsum[:])
                else:
                    nc.vector.tensor_add(out=fmax[:], in0=fmax[:], in1=lmax[:])
                    nc.vector.tensor_add(out=fsum[:], in0=fsum[:], in1=lsum[:])

                nxt = {}
                if li < 2 and int(os.environ.get("GCN_LAYERS", "3")) > li + 1:
                    # peel: next layer's first-group seg-0 gathers can run
                    # during the tail AllGather (their source AG already done)
                    for pg in (0, 1):
                        for ci, h, pos, k in calls_of_group.get(pg, []):
                            if h == 0:
                                emit_call(li + 1, ci, h, k, nxt)
                    s = NSEG - 1
                    nc.gpsimd.collective_compute(
                        "AllGather", mybir.AluOpType.bypass,
                        replica_groups=rg,
                        ins=[h_mine[li][SEGB[s] * P:SEGB[s + 1] * P, :]],
                        outs=[h_full[li][s][:, :]],
                    )
                return nxt

            _pre = {}
            for _li in range(int(os.environ.get("GCN_LAYERS", "3"))):
                _pre = layer(_li, _pre)

            nc.vector.tensor_scalar_mul(fsum[:], fsum[:],
                                        float(np.float32(1.0 / cfg.GRAPH)))
            osb = workp.tile([P, 2 * NS * cfg.GPC], F32, tag="osb")
            nc.vector.tensor_copy(out=osb[:, :NS * cfg.GPC], in_=fmax[:])
            nc.vector.tensor_copy(out=osb[:, NS * cfg.GPC:], in_=fsum[:])
            nc.sync.dma_start(out=out[:], in_=osb[:])

    nc.compile()
    return nc


def unshard(cfg, results):
    """[NCORES][128, 2*NS*GPC] -> [B, 2*COUT, T] float32."""
    B, T, COUT, NS, GPC = cfg.B, cfg.T, cfg.COUT, cfg.NS, cfg.GPC
    out = np.zeros((B, 2 * COUT, T), np.float32)
    for c in range(cfg.NCORES):
        V = results[c]["out"]
        for gl in range(cfg.gpc[c]):
            g = cfg.goff[c] + gl
            for s in range(NS):
                for half in range(2):
                    t_ = 2 * s + half
                    co = np.arange(COUT)
                    pp = half * COUT + co
                    out[g, co, t_] = V[pp, s * GPC + gl]
                    out[g, COUT + co, t_] = V[pp, NS * GPC + s * GPC + gl]
    return out


_CACHE = {}


def kernel(**inputs):
    cfg = Cfg()
    common, per_core, meta = preprocess(
        cfg, inputs["x"], inputs["edge_index"], inputs["batch"],
        inputs["W1"], inputs["b1"], inputs["W2"], inputs["b2"],
        inputs["W3"], inputs["b3"])
    key = (tuple(meta["Kbh"].flatten().tolist()), meta["NCALLS"])
    if key not in _CACHE:
        _CACHE[key] = build(cfg, meta)
    nc = _CACHE[key]
    in_maps = []
    for c in range(cfg.NCORES):
        m = dict(common)
        m.update(per_core[c])
        in_maps.append(m)
    res = run_bass_kernel_spmd(nc, in_maps, list(range(cfg.NCORES)))
    return unshard(cfg, res.results)
